# revision 63
# baseline (speedup 1.0000x reference)
"""Trainium2 Bass kernel for nn_CaduceusEmbeddingsSTFT.

out[b, t, :] = concat(emb_table[ids[b, t]],
                      proj(|STFT(onehot(ids[b]))| upsampled at frame f(t)))

Structure exploited:
  * nearest upsampling -> only 65 distinct STFT frame rows per core-half;
    the projection collapses to (65 x 2064) @ (2064 x 154).
  * STFT of one-hot signals: windowed frames are one-hot masks, so
    spec = onehot_frames @ (window * DFT) as matmuls (cos / sin).
  * every output row is concat(emb_row[id(t)], S[frame(t)]) -- built by a
    SINGLE matmul per 128-row tile: lhsT stacks the id one-hot (16 rows)
    and the frame one-hot (65 rows), rhs is the table [emb | S].

Precision: harness gate is rel_err < 2e-2; everything runs plain bf16
(fp32 PSUM accumulation), output DMA'd as bf16 and upcast on host.
Measured numpy sim of this scheme: rel err ~4.8e-3.

Sharding: 8 cores = 4 batches x 2 sequence halves; each core computes a
(4096, 512) output shard; boundary frame recomputed by both halves.

Perf design (per core), measured ~42-44us NEFF total:
  * inputs on 3 concurrent DMA paths: cw bf16 + proj weights on the Sync
    HWDGE ring, rhs-table image + nyquist row on the Scalar HWDGE ring,
    both one-hot payloads as int8 with SWDGE casting to bf16 in flight.
  * ~26 dummy matmuls pre-warm the PE HAM clock gate (cold = 1.2 GHz,
    warm = 2.4 GHz) while the input DMAs land; a tiny early sqrt pins
    the ACT table load into the same dead zone.
  * DFT (6 streams x 1040 cols, K=128) -> |mag| (squares ACT, add DVE,
    sqrt ACT; ACT-serial, the S critical path) -> projection (16 K=128
    MMs + 16 K=1 nyquist MMs per chunk) -> bias via the preloaded table
    image -> 32 fused N=512 output MMs.
  * output rows permuted so each partition holds 4 CONSECUTIVE HBM rows:
    4KB DMA descriptors stream at ~300 GB/s (1KB descriptors measured
    104 GB/s); drains alternate DVE/ACT; out DMAs alternate both HWDGE
    rings; keep-warm filler MMs bridge PE idle gaps so HAM stays at 8/8.
Remaining structure: ~7us Tile entry + ~10us exit barriers are fixed;
the 14us output DMA stream and the ACT-serial mag chain pace the rest.
"""

import numpy as np

V = 16
D_EMB = 358
D_STFT = 154
NFFT = 256
HOP = 64
NFREQ = 129
B, L = 4, 8192
LH = L // 2  # 4096 rows per core
F = 65  # frames per core (inclusive overlap frame)
VF = V * F  # 1040
DM = 512
NCORES = 8
NT = LH // 128  # 32 output tiles per core
NQ = NT // 4  # q-groups of 4 tiles
KOUT = V + F  # 81: stacked one-hot rows in the output matmul
CWW = 2 * NFREQ  # 258: per-c block width in cw (cos 0..127 | ny | sin 0..127)
# (start, size) chunks over the VF axis; multiples of F so projection
# lhsT slices [:, v*F:(v+1)*F] never cross a chunk boundary; <=512 f32
# per PSUM bank.
CHUNKS = [(0, 7 * F), (7 * F, 7 * F), (14 * F, 2 * F)]
NDUM = 26  # PE warm-up matmuls issued while input DMAs land
NFILL2 = 2  # keep-warm matmuls ready with chunk-2 |mag|
NFILL3 = 2  # keep-warm matmuls ready with chunk-3 |mag|

_PROG = None
LAST_RESULT = None  # BassKernelResults of the most recent run (for harnesses)


def _build_program():
    import concourse.mybir as mybir
    import concourse.tile as tile
    from concourse import bacc

    f32 = mybir.dt.float32
    bf16 = mybir.dt.bfloat16
    AO = mybir.AluOpType
    AF = mybir.ActivationFunctionType

    nc = bacc.Bacc("TRN2", target_bir_lowering=False, debug=False,
                   num_devices=NCORES)

    i8 = mybir.dt.int8
    cwt = nc.dram_tensor("cwt", [128, 2 * CWW], bf16, kind="ExternalInput")
    ohf8 = nc.dram_tensor("ohf8", [128, 2 * VF], i8, kind="ExternalInput")
    pk2 = nc.dram_tensor("pk2", [128, V * D_STFT], bf16, kind="ExternalInput")
    nyw = nc.dram_tensor("nyw", [1, V * D_STFT], bf16, kind="ExternalInput")
    rtim = nc.dram_tensor("rtim", [KOUT, DM], bf16, kind="ExternalInput")
    bfsel = nc.dram_tensor("bfsel", [KOUT, LH], i8, kind="ExternalInput")
    out = nc.dram_tensor("out", [LH, DM], bf16, kind="ExternalOutput")

    with tile.TileContext(nc) as tc:
        with (
            tc.tile_pool(name="consts", bufs=1) as cpool,
            tc.tile_pool(name="work", bufs=1) as wpool,
            tc.tile_pool(name="tmp", bufs=2) as tpool,
            tc.tile_pool(name="ostg", bufs=3) as ospool,
        ):
            # ---- const loads: 3 concurrent paths, S-critical bytes first ---
            # Sync HWDGE ring: cw (tiny, DFT weights) then wproj (gates the
            # projection -> S -> everything). SWDGE queue: both one-hot
            # payloads as int8, cast to bf16 in flight (half the HBM bytes).
            # Scalar HWDGE ring: rhs-table image + nyquist row (small).
            CWT = cpool.tile([128, 2 * CWW], bf16, tag="cwt")
            nc.sync.dma_start(out=CWT[:], in_=cwt[:])
            OHFT = cpool.tile([128, 2 * VF], bf16, tag="ohft")
            nc.gpsimd.dma_start(out=OHFT[:], in_=ohf8[:])
            PK2 = cpool.tile([128, V * D_STFT], bf16, tag="pk2")
            nc.sync.dma_start(out=PK2[:], in_=pk2[:])
            RTIM = cpool.tile([KOUT, DM], bf16, tag="rtim")
            nc.scalar.dma_start(out=RTIM[:], in_=rtim[:])
            NYW = cpool.tile([1, V * D_STFT], bf16, tag="nyw")
            nc.scalar.dma_start(out=NYW[:], in_=nyw[:])
            BF = cpool.tile([KOUT, LH], bf16, tag="bf")
            nc.gpsimd.dma_start(out=BF[:], in_=bfsel[:])

            OHF = [OHFT[:, 0:VF], OHFT[:, VF:2 * VF]]
            CW = CWT
            WP = PK2
            WNR = NYW

            # ---- on-chip work tiles ----------------------------------------
            ZW = wpool.tile([128, 128], bf16, tag="zw")
            nc.vector.memset(ZW[:], 0.0)
            # tiny sqrt first in ACT program order: walrus resolves ACT
            # table sets greedily at the first ACTIVATE needing one, and
            # the sqrt set also contains square/abs -- this pins the
            # 1.3us ACT_TABLE_LOAD into the input-DMA dead zone instead
            # of the middle of the mag chain.
            SCR = wpool.tile([1, 8], f32, tag="scr")
            with tc.high_priority():
                nc.scalar.sqrt(out=SCR[:], in_=ZW[0:1, 0:8])
            # rhs table init: rows 0..64 [0 | bias], rows 65..80 [emb | 0]
            RT = wpool.tile([KOUT, DM], bf16, tag="rt")
            nc.vector.tensor_copy(out=RT[:], in_=RTIM[:])
            MAGH = wpool.tile([128, VF], bf16, tag="magh")
            NYB = wpool.tile([1, VF], bf16, tag="nyb")

            with tc.tile_pool(name="psum_s", bufs=1, space="PSUM") as psp:
                S = psp.tile([F, D_STFT], f32, tag="s")

                with (
                    tc.tile_pool(name="psum_dum", bufs=1, space="PSUM") as pdm,
                    tc.tile_pool(name="psum_re", bufs=2, space="PSUM") as pre,
                    tc.tile_pool(name="psum_im", bufs=2, space="PSUM") as pim,
                    tc.tile_pool(name="psum_ny", bufs=2, space="PSUM") as pny,
                ):
                    # PE warm-up: no input deps, scheduler runs these first;
                    # ~18 x (ldw+mm) ~= 3.4us busy -> HAM releases to 2.4 GHz
                    # right as the first real matmul's data lands.
                    DU = pdm.tile([128, 128], f32, tag="du")
                    for _ in range(NDUM):
                        nc.tensor.matmul(out=DU[:], lhsT=ZW[:], rhs=ZW[:],
                                         start=True, stop=True)

                    first_s = [True]

                    def proj_mm(lhsT, rhs, stop=False):
                        nc.tensor.matmul(out=S[:], lhsT=lhsT, rhs=rhs,
                                         start=first_s[0], stop=stop)
                        first_s[0] = False

                    for c0, cn in CHUNKS:
                        re = pre.tile([128, 7 * F], f32, tag="re")
                        im = pim.tile([128, 7 * F], f32, tag="im")
                        ny = pny.tile([1, 7 * F], f32, tag="ny")
                        for c in range(2):
                            cb = c * CWW
                            rhs = OHF[c][:, c0:c0 + cn]
                            nc.tensor.matmul(
                                out=re[:, :cn], lhsT=CW[:, cb:cb + 128],
                                rhs=rhs, start=(c == 0), stop=(c == 1))
                            nc.tensor.matmul(
                                out=im[:, :cn],
                                lhsT=CW[:, cb + NFREQ:cb + NFREQ + 128],
                                rhs=rhs, start=(c == 0), stop=(c == 1))
                            nc.tensor.matmul(
                                out=ny[:, :cn], lhsT=CW[:, cb + 128:cb + 129],
                                rhs=rhs, start=(c == 0), stop=(c == 1))
                        with tc.high_priority():
                            # |spec| = sqrt(re^2 + im^2); squares on ACT (DVE
                            # cannot read two PSUM operands)
                            t1 = tpool.tile([128, 7 * F], f32, tag="sq1")
                            t2 = tpool.tile([128, 7 * F], f32, tag="sq2")
                            nc.scalar.square(out=t1[:, :cn], in_=re[:, :cn])
                            nc.scalar.square(out=t2[:, :cn], in_=im[:, :cn])
                            nc.vector.tensor_tensor(
                                out=t1[:, :cn], in0=t1[:, :cn], in1=t2[:, :cn],
                                op=AO.add)
                            nc.scalar.sqrt(out=MAGH[:, c0:c0 + cn],
                                           in_=t1[:, :cn])
                            nc.scalar.activation(NYB[:, c0:c0 + cn],
                                                 ny[:, :cn], AF.Abs)
                            for v in range(c0 // F, (c0 + cn) // F):
                                proj_mm(MAGH[:, v * F:(v + 1) * F],
                                        WP[:, v * D_STFT:(v + 1) * D_STFT])
                            # nyquist bin: tiny K=1 matmuls off this chunk's
                            # |ny| slice -- distributed per chunk so the S
                            # tail after the last sqrt is only ~0.7us
                            for v in range(c0 // F, (c0 + cn) // F):
                                proj_mm(NYB[0:1, v * F:(v + 1) * F],
                                        WNR[0:1, v * D_STFT:(v + 1) * D_STFT],
                                        stop=(c0 + cn == VF
                                              and v == VF // F - 1))

                    # keep-warm fillers: become ready with chunk-2/3 |mag|,
                    # so the scheduler slots them into the PE idle window
                    # while the S tail runs on ACT/DVE. Without these the
                    # ~2.5us gap re-throttles HAM and the whole output phase
                    # runs at 1.2 GHz.
                    for _ in range(NFILL2):
                        nc.tensor.matmul(out=DU[:],
                                         lhsT=MAGH[:, 14 * F - 128:14 * F],
                                         rhs=ZW[:], start=True, stop=True)
                    for _ in range(NFILL3):
                        nc.tensor.matmul(out=DU[:],
                                         lhsT=MAGH[:, VF - 128:VF],
                                         rhs=ZW[:], start=True, stop=True)

                with tc.high_priority():
                    # rhs table rows 0..64 = S + bias image (in-place add)
                    nc.vector.tensor_tensor(
                        out=RT[0:F, D_EMB:DM], in0=S[:],
                        in1=RT[0:F, D_EMB:DM], op=AO.add)

            # ---- output: one fused matmul per 128-row tile ------------------
            # MMs (~216ns) outpace the DVE/ACT drains (~330ns/tile): the PE
            # micro-idles once PSUM fills, which re-throttles HAM mid-phase.
            # Two fillers per q-group (gated on this group's drains) keep
            # its activity window busy.
            with (
                tc.tile_pool(name="psum_out", bufs=7, space="PSUM") as pout,
                tc.tile_pool(name="psum_fil", bufs=1, space="PSUM") as pfil,
            ):
                for q in range(NQ):
                    os_ = ospool.tile([128, 4 * DM], bf16, tag="os")
                    for a in range(4):
                        # partition p of sub-tile a holds output row
                        # 512q + 4p + a, so each partition's 4 rows are
                        # CONSECUTIVE in HBM -> 4KB DMA descriptors instead
                        # of 1KB (1KB descs measured at only ~104 GB/s)
                        po = pout.tile([128, DM], f32, tag="po")
                        c0_ = 512 * q + a
                        nc.tensor.matmul(
                            out=po[:], lhsT=BF[:, c0_:c0_ + 509:4],
                            rhs=RT[0:KOUT, :], start=True, stop=True)
                        sl = os_[:, a * DM:(a + 1) * DM]
                        if a % 2 == 0:
                            nc.vector.tensor_copy(out=sl, in_=po[:])
                        else:
                            nc.scalar.copy(out=sl, in_=po[:])
                    DU2 = pfil.tile([128, 64], f32, tag="fil")
                    nc.tensor.matmul(out=DU2[:], lhsT=os_[:, 0:128],
                                     rhs=ZW[:, 0:64], start=True, stop=True)
                    nc.tensor.matmul(out=DU2[:], lhsT=os_[:, DM:DM + 128],
                                     rhs=ZW[:, 0:64], start=True, stop=True)
                    eng = nc.sync if q % 2 == 0 else nc.scalar
                    eng.dma_start(
                        out=out[q * 512:(q + 1) * 512, :]
                        .rearrange("(p a) e -> p a e", a=4),
                        in_=os_[:].rearrange("p (a e) -> p a e", a=4))

    nc.finalize()
    return nc


def _host_consts():
    import ml_dtypes

    bf16 = ml_dtypes.bfloat16
    n = np.arange(NFFT)
    window = 0.5 - 0.5 * np.cos(2.0 * np.pi * n / NFFT)
    k = np.arange(NFREQ)
    ang = 2.0 * np.pi * np.outer(n, k) / NFFT  # (256, 129)
    wcos = (window[:, None] * np.cos(ang)).astype(np.float32)
    wsin = (window[:, None] * np.sin(ang)).astype(np.float32)
    cw = np.zeros((128, 2 * CWW), np.float32)
    for c in range(2):
        rows = slice(c * 128, (c + 1) * 128)
        blk = np.zeros((128, CWW), np.float32)
        blk[:, :128] = wcos[rows, :128]
        blk[:, 128] = wcos[rows][:, 128]  # nyquist cos column
        blk[:, NFREQ:NFREQ + 128] = wsin[rows, :128]
        cw[:, c * CWW:(c + 1) * CWW] = blk
    return cw.astype(bf16)


def kernel(input_ids, emb_table, proj_w, proj_b):
    global _PROG, LAST_RESULT
    import ml_dtypes

    from concourse.bass_utils import run_bass_kernel_spmd

    bf16 = ml_dtypes.bfloat16
    ids = np.asarray(input_ids).astype(np.int64)
    emb = np.asarray(emb_table).astype(np.float32)
    pw = np.asarray(proj_w).astype(np.float32)
    pb = np.asarray(proj_b).astype(np.float32)

    cw = _host_consts()

    # pk1 cols [2*VF:] = cw; per-core ohf fills cols [:2*VF]
    # pk2: proj weights, rows k=0..127, cols v*154+o  (proj_w row i=k*V+v)
    pk2 = np.zeros((128, V * D_STFT), np.float32)
    for v in range(V):
        pk2[:, v * D_STFT:(v + 1) * D_STFT] = pw[np.arange(128) * V + v]
    pk2 = pk2.astype(bf16)

    # nyw: nyquist-bin proj weights as a single partition-0 row
    nywr = np.zeros((1, V * D_STFT), np.float32)
    for v in range(V):
        nywr[0, v * D_STFT:(v + 1) * D_STFT] = pw[128 * V + v]
    nywr = nywr.astype(bf16)

    # rhs-table init image: rows 0..64 [0 | bias], rows 65..80 [emb | 0]
    rtimg = np.zeros((KOUT, DM), np.float32)
    rtimg[:F, D_EMB:] = pb[None, :]
    rtimg[F:, :D_EMB] = emb

    vr = np.arange(V)
    in_maps = []
    for core in range(NCORES):
        b, h = divmod(core, 2)
        padded = np.pad(ids[b], 128, mode="reflect")
        seg = padded[LH * h:LH * h + 64 * (F - 1) + NFFT]  # (4352,)
        ohf = np.zeros((128, 2 * VF), np.int8)
        for c in range(2):
            sv = seg[(128 * c + np.arange(128))[:, None]
                     + 64 * np.arange(F)[None, :]]  # (128, F)
            oh = (sv[:, None, :] == vr[None, :, None])  # (128, V, F)
            ohf[:, c * VF:(c + 1) * VF] = oh.reshape(128, VF)

        ids_h = ids[b, LH * h:LH * (h + 1)]
        t = np.arange(LH)
        floc = ((129 * (t + LH * h)) >> 13) - 64 * h
        bf = np.zeros((KOUT, LH), np.int8)
        bf[floc, t] = 1
        bf[F + ids_h, t] = 1
        in_maps.append({
            "cwt": cw, "ohf8": ohf, "pk2": pk2, "nyw": nywr,
            "rtim": rtimg.astype(bf16), "bfsel": bf,
        })

    if _PROG is None:
        _PROG = _build_program()

    res = run_bass_kernel_spmd(_PROG, in_maps, core_ids=list(range(NCORES)))
    LAST_RESULT = res

    full = np.zeros((B, L, DM), np.float32)
    for core in range(NCORES):
        b, h = divmod(core, 2)
        full[b, LH * h:LH * (h + 1), :] = \
            res.results[core]["out"].astype(np.float32)
    return full
